# revision 1
# baseline (speedup 1.0000x reference)
"""Trainium2 Bass kernel for nn_Cell2Cell (retrieval_knn, 4-head Markov power).

Sharding: head-parallel x row-parallel. Core c -> head h=c//2, half=c%2.
Each core: per-head projections (fp32r matmuls), row-block distance matrix via
augmented-gram matmul (qq/kk norms folded in as extra contraction rows), exact
per-row rank-11/rank-30 selection with DVE max8+match_replace, knn mask in aff
domain, symmetrization via a transposed-gram pass (no transposes anywhere),
E=exp(S-2) with fused row-sum, pair AllGather of E and Z, then 6 power
iterations column-split over V with invZ folded into the PSUM eviction scale.
Host sums head partials for the mean.
"""
import sys
sys.path.insert(0, '/opt/trn_rl_repo')
import numpy as np

N = 4096
D = 2048
HID = 256
HEADS = 4
T_POWER = 6
NCORES = 8
HALF = N // 2          # 2048 rows per core
VCOL = D // 2          # 1024 V-columns per core
RT = HALF // 128       # 16 row tiles per core
KT = HID // 128        # 2 hidden k-tiles
DKT = D // 128         # 16 input-dim k-tiles

_CACHE = {}


def _build(sim=False):
    import concourse.bacc as bacc
    import concourse.mybir as mybir
    import concourse.tile as tile

    dt = mybir.dt
    AF = mybir.ActivationFunctionType
    OP = mybir.AluOpType

    nc = bacc.Bacc("TRN2", target_bir_lowering=False, debug=False,
                   num_devices=1 if sim else NCORES)

    f32, f32r = dt.float32, dt.float32r

    # ---------------- I/O ----------------
    xt = nc.dram_tensor("xt", [D, N], f32, kind="ExternalInput")        # X.T
    xt_own = nc.dram_tensor("xt_own", [D, HALF], f32, kind="ExternalInput")
    xcol = nc.dram_tensor("xcol", [N, VCOL], f32, kind="ExternalInput")
    wqt = nc.dram_tensor("wqt", [D, HID], f32, kind="ExternalInput")    # Wq[h].T
    wkt = nc.dram_tensor("wkt", [D, HID], f32, kind="ExternalInput")
    bqc = nc.dram_tensor("bqc", [HID, 1], f32, kind="ExternalInput")
    bkc = nc.dram_tensor("bkc", [HID, 1], f32, kind="ExternalInput")
    e2a = nc.dram_tensor("e2a", [128, 128], f32, kind="ExternalInput")  # 2I or 0
    ema = nc.dram_tensor("ema", [128, 128], f32, kind="ExternalInput")  # 1-I or 1
    e2b = nc.dram_tensor("e2b", [128, 128], f32, kind="ExternalInput")
    emb = nc.dram_tensor("emb", [128, 128], f32, kind="ExternalInput")
    out = nc.dram_tensor("out", [N, VCOL], f32, kind="ExternalOutput")

    PAIRS = [[0, 1], [2, 3], [4, 5], [6, 7]]

    with tile.TileContext(nc) as tc:
        with (
            tc.tile_pool(name="persist", bufs=1) as pp,
            tc.tile_pool(name="dram", bufs=1, space="DRAM") as dram,
        ):
            # ---- persistent DRAM buffers ----
            a_own = dram.tile([HALF, N], f32)            # masked affA rows
            e_own = dram.tile([HALF, N], f32r)
            e_full = dram.tile([N, N], f32r)
            st_in = dram.tile([2, HALF], f32)            # [invmd2; kth]
            st_out = dram.tile([4, HALF], f32)
            z_own = dram.tile([HALF, 1], f32)
            z_full = dram.tile([N, 1], f32)
            vbuf0 = dram.tile([N, VCOL], f32r)
            vbuf1 = dram.tile([N, VCOL], f32r)

            # ---- small persistent SBUF ----
            b1e10 = pp.tile([128, 1], f32)
            nc.vector.memset(b1e10[:], 1e-10)
            bneg2 = pp.tile([128, 1], f32)
            nc.vector.memset(bneg2[:], -2.0)
            ones_f = pp.tile([128, 1], f32)
            nc.vector.memset(ones_f[:], 1.0)
            ones_l = pp.tile([128, 1], f32r)
            nc.vector.tensor_copy(ones_l[:], ones_f[:])
            eye2a = pp.tile([128, 128], f32)
            eyema = pp.tile([128, 128], f32)
            eye2b = pp.tile([128, 128], f32)
            eyemb = pp.tile([128, 128], f32)
            nc.sync.dma_start(eye2a[:], e2a[:, :])
            nc.sync.dma_start(eyema[:], ema[:, :])
            nc.sync.dma_start(eye2b[:], e2b[:, :])
            nc.sync.dma_start(eyemb[:], emb[:, :])

            qtf_d = dram.tile([128, KT * N], f32r)
            k2o_d = dram.tile([128, KT * HALF], f32r)
            aglt_d = dram.tile([2, HALF], f32r)
            agrt_d = dram.tile([2, N], f32r)
            pjb_cm = tc.tile_pool(name="projsB", bufs=1)
            pjb = pjb_cm.__enter__()                   # live P0..P1
            if True:
                ktf = pjb.tile([128, KT, N], f32r)     # kT_full
                q2o = pjb.tile([128, KT, HALF], f32r)  # 2*qT_own
                agl_a = pjb.tile([2, HALF], f32r)      # [-qq_own; -1]
                agr_a = pjb.tile([2, N], f32r)         # [1; kk_full]
                pja_cm = tc.tile_pool(name="projsA", bufs=1)
                pja = pja_cm.__enter__()               # live P0 only (spilled)
                qtf = pja.tile([128, KT, N], f32r)     # qT_full
                k2o = pja.tile([128, KT, HALF], f32r)  # 2*kT_own
                agl_t = pja.tile([2, HALF], f32r)      # [-kk_own; -1]
                agr_t = pja.tile([2, N], f32r)         # [1; qq_full]

                # ================= P0: projections =================
                with (
                    tc.tile_pool(name="p0", bufs=2) as p0,
                    tc.tile_pool(name="p0w", bufs=1) as p0w,
                    tc.tile_pool(name="ps0", bufs=2, space="PSUM") as ps0,
                ):
                    wq_s = p0w.tile([128, DKT, HID], f32r)
                    wk_s = p0w.tile([128, DKT, HID], f32r)
                    for wsrc, wdst in ((wqt, wq_s), (wkt, wk_s)):
                        wr = wsrc.ap().rearrange("(a p) m -> p a m", p=128)
                        for ch in range(2):
                            wf = p0.tile([128, DKT // 2, HID], f32,
                                         tag="wstg", bufs=1,
                                         name=f"wf_{wdst.tensor.name}_{ch}")
                            nc.sync.dma_start(
                                wf[:], wr[:, ch * 8:(ch + 1) * 8, :])
                            nc.vector.tensor_copy(
                                wdst[:, ch * 8:(ch + 1) * 8, :], wf[:])
                    bq_s = p0w.tile([128, KT], f32)
                    bk_s = p0w.tile([128, KT], f32)
                    nc.sync.dma_start(
                        bq_s[:], bqc.ap().rearrange("(a p) o -> p (a o)", p=128))
                    nc.sync.dma_start(
                        bk_s[:], bkc.ap().rearrange("(a p) o -> p (a o)", p=128))

                    xt_r = xt.ap().rearrange("(a p) n -> p a n", p=128)
                    xto_r = xt_own.ap().rearrange("(a p) n -> p a n", p=128)

                    def proj(nb, rhs_src, pairs):
                        # kk-outer: one rhs k-tile shared by all 4 psums
                        psms = []
                        for w_s, b_s, dst, scaled in pairs:
                            for mt in range(KT):
                                psms.append(ps0.tile(
                                    [128, 512], f32, tag=f"psm{len(psms)}",
                                    name=f"psm{nb}_{len(psms)}"))
                        for kk in range(DKT):
                            slf = p0.tile([128, 512], f32, tag="rhsf",
                                          bufs=3, name=f"rhsf{nb}_{kk}")
                            nc.sync.dma_start(
                                slf[:], rhs_src[:, kk,
                                                nb * 512:(nb + 1) * 512])
                            sl = p0.tile([128, 512], f32r, tag="rhs",
                                         bufs=3, name=f"rhs{nb}_{kk}")
                            nc.vector.tensor_copy(sl[:], slf[:])
                            i = 0
                            for w_s, b_s, dst, scaled in pairs:
                                for mt in range(KT):
                                    nc.tensor.matmul(
                                        psms[i],
                                        w_s[:, kk, mt * 128:(mt + 1) * 128],
                                        sl[:],
                                        start=(kk == 0), stop=(kk == DKT - 1))
                                    i += 1
                        i = 0
                        for w_s, b_s, dst, scaled in pairs:
                            for mt in range(KT):
                                if scaled:
                                    nc.vector.tensor_scalar(
                                        dst[:, mt, nb * 512:(nb + 1) * 512],
                                        psms[i], b_s[:, mt:mt + 1], 2.0,
                                        OP.add, OP.mult)
                                else:
                                    nc.vector.tensor_scalar_add(
                                        dst[:, mt, nb * 512:(nb + 1) * 512],
                                        psms[i], b_s[:, mt:mt + 1])
                                i += 1

                    for nb in range(N // 512):
                        proj(nb, xt_r, ((wq_s, bq_s, qtf, False),
                                        (wk_s, bk_s, ktf, False)))
                    for nb in range(HALF // 512):
                        proj(nb, xto_r, ((wq_s, bq_s, q2o, True),
                                         (wk_s, bk_s, k2o, True)))

                # ---- norms via ones-matmul over squared projections ----
                with (
                    tc.tile_pool(name="pn", bufs=1) as pn,
                    tc.tile_pool(name="psn", bufs=4, space="PSUM") as psn,
                ):
                    trow = pn.tile([1, 512], f32r, tag="trow")
                    cm = pn.tile([2, N], f32, tag="cm")
                    nc.vector.memset(cm[:, :], -1.0)
                    nc.vector.tensor_copy(agl_a[:, :], cm[:, :HALF])
                    nc.vector.tensor_copy(agl_t[:, :], cm[:, :HALF])
                    nc.vector.memset(cm[:, :], 1.0)
                    nc.vector.tensor_copy(agr_a[:, :], cm[:, :])
                    nc.vector.tensor_copy(agr_t[:, :], cm[:, :])
                    for src, aug, row, sgn, w in (
                        (ktf, agr_a, 1, 1.0, N),       # +kk_full
                        (qtf, agr_t, 1, 1.0, N),       # +qq_full
                        (q2o, agl_a, 0, -0.25, HALF),  # -qq_own (q2o = 2q)
                        (k2o, agl_t, 0, -0.25, HALF),  # -kk_own
                    ):
                        sq = pn.tile([128, KT, N], f32r, tag="sq",
                                     name=f"sq_{aug.tensor.name}_{row}")
                        nc.vector.tensor_tensor(
                            sq[:, :, :w], src[:, :, :w], src[:, :, :w], OP.mult)
                        for nb in range(w // 512):
                            pst = psn.tile([1, 512], f32, tag="pst",
                                           name=f"pst{nb}")
                            for kt in range(KT):
                                nc.tensor.matmul(
                                    pst[:], ones_l[:],
                                    sq[:, kt, nb * 512:(nb + 1) * 512],
                                    start=(kt == 0), stop=(kt == KT - 1))
                            if row == 0:
                                nc.vector.tensor_scalar_mul(
                                    aug[0:1, nb * 512:(nb + 1) * 512], pst[:], sgn)
                            else:
                                tr = pn.tile([1, 512], f32r, tag="trow",
                                             name=f"tr_{aug.tensor.name}_{nb}")
                                nc.vector.tensor_scalar_mul(tr[:], pst[:], sgn)
                                nc.sync.dma_start(
                                    aug[1:2, nb * 512:(nb + 1) * 512], tr[:])

                # ---- spill P3-only tensors, free their SBUF ----
                nc.sync.dma_start(qtf_d[:, :], qtf.rearrange("p a n -> p (a n)"))
                nc.sync.dma_start(k2o_d[:, :], k2o.rearrange("p a n -> p (a n)"))
                nc.sync.dma_start(aglt_d[:, :], agl_t[:, :])
                nc.sync.dma_start(agrt_d[:, :], agr_t[:, :])
                pja_cm.__exit__(None, None, None)

                # ================= P1: A-side rows + stats =================
                with (
                    tc.tile_pool(name="big1", bufs=8) as pb,
                    tc.tile_pool(name="pbs1", bufs=2) as pbs,
                    tc.tile_pool(name="ps1", bufs=1, space="PSUM") as ps1,
                ):
                    p1, p1s = pb, pbs
                    prev = None  # (msk, im2, kth, r0, r1) delayed by one tile
                    for rt in range(RT):
                        r0, r1 = rt * 128, (rt + 1) * 128
                        nsq = p1.tile([128, N], f32, tag="big",
                                      name=f"nsq{rt}")
                        psg = ps1.tile([128, N], f32, tag="psg",
                                       name=f"psg{rt}")
                        for nb in range(N // 512):
                            pslc = psg[:, nb * 512:(nb + 1) * 512]
                            for kt in range(KT):
                                nc.tensor.matmul(
                                    pslc, q2o[:, kt, r0:r1],
                                    ktf[:, kt, nb * 512:(nb + 1) * 512],
                                    start=(kt == 0), stop=False)
                            nc.tensor.matmul(
                                pslc, agl_a[:, r0:r1],
                                agr_a[:, nb * 512:(nb + 1) * 512],
                                start=False, stop=True)
                        nc.scalar.copy(nsq[:], psg[:])
                        # exact 32 smallest sq = 32 largest of nsq (=-sq)
                        sel = p1s.tile([128, 32], f32, tag="sel",
                                       name=f"sel{rt}")
                        sca = p1.tile([128, N], f32, tag="big",
                                      name=f"sca{rt}")
                        nc.vector.max(sel[:, 0:8], nsq[:])
                        nc.vector.match_replace(sca[:], sel[:, 0:8], nsq[:],
                                                -1e30)
                        scb = p1.tile([128, N], f32, tag="big",
                                      name=f"scb{rt}")
                        nc.vector.max(sel[:, 8:16], sca[:])
                        nc.vector.match_replace(scb[:], sel[:, 8:16], sca[:],
                                                -1e30)
                        scc = p1.tile([128, N], f32, tag="big",
                                      name=f"scc{rt}")
                        nc.vector.max(sel[:, 16:24], scb[:])
                        nc.vector.match_replace(scc[:], sel[:, 16:24], scb[:],
                                                -1e30)
                        nc.vector.max(sel[:, 24:32], scc[:])
                        # stats on DVE: im2 = 1/relu(sq11), kth = exp(-sq30*im2)
                        t11 = p1s.tile([128, 1], f32, tag="t11",
                                       name=f"t11{rt}")
                        nc.vector.tensor_scalar(t11[:], sel[:, 10:11], -1.0,
                                                1e-20, OP.mult, OP.max)
                        im2 = p1s.tile([128, 1], f32, tag="im2",
                                       name=f"im2{rt}")
                        nc.vector.reciprocal(im2[:], t11[:])
                        kth = p1s.tile([128, 1], f32, tag="kth",
                                       name=f"kth{rt}")
                        nc.scalar.activation(kth[:], sel[:, 29:30], AF.Exp,
                                             scale=im2[:, 0:1])
                        # aff = exp(nsq * im2)   (nsq = -sq)
                        aff = p1.tile([128, N], f32, tag="big",
                                      name=f"aff{rt}")
                        nc.scalar.activation(aff[:], nsq[:], AF.Exp,
                                             scale=im2[:, 0:1])
                        if prev is not None:
                            paff, pim2, pkth, pr0, pr1 = prev
                            pmsk = p1.tile([128, N], f32, tag="big",
                                           name=f"msk{rt - 1}")
                            nc.vector.scalar_tensor_tensor(
                                pmsk[:], paff[:], pkth[:, 0:1], paff[:],
                                op0=OP.is_ge, op1=OP.mult)
                            nc.sync.dma_start(a_own[pr0:pr1, :], pmsk[:])
                            nc.sync.dma_start(st_in[0:1, pr0:pr1], pim2[:])
                            nc.sync.dma_start(st_in[1:2, pr0:pr1], pkth[:])
                        prev = (aff, im2, kth, r0, r1)
                    paff, pim2, pkth, pr0, pr1 = prev
                    pmsk = p1.tile([128, N], f32, tag="big", name="msk_last")
                    nc.vector.scalar_tensor_tensor(
                        pmsk[:], paff[:], pkth[:, 0:1], paff[:],
                        op0=OP.is_ge, op1=OP.mult)
                    nc.sync.dma_start(a_own[pr0:pr1, :], pmsk[:])
                    nc.sync.dma_start(st_in[0:1, pr0:pr1], pim2[:])
                    nc.sync.dma_start(st_in[1:2, pr0:pr1], pkth[:])

                pjb_cm.__exit__(None, None, None)

                # ============ P2: stats allgather + bcast mats ============
                if sim:
                    nc.sync.dma_start(st_out[0:2, :], st_in[:, :])
                    nc.sync.dma_start(st_out[2:4, :], st_in[:, :])
                else:
                    nc.gpsimd.collective_compute(
                        "AllGather", OP.bypass, replica_groups=PAIRS,
                        ins=[st_in.opt()], outs=[st_out.opt()])

                # ================= P3: AT-side + S + E =====================
                with (
                    tc.tile_pool(name="rl", bufs=1) as rl,
                    tc.tile_pool(name="mats", bufs=1) as pm,
                    tc.tile_pool(name="big3", bufs=6) as pb3,
                    tc.tile_pool(name="pbs3", bufs=2) as pbs,
                    tc.tile_pool(name="ps3", bufs=1, space="PSUM") as ps3,
                ):
                    p3 = pb3
                    qtf = rl.tile([128, KT, N], f32r)
                    k2o = rl.tile([128, KT, HALF], f32r)
                    agl_t = rl.tile([2, HALF], f32r)
                    agr_t = rl.tile([2, N], f32r)
                    nc.sync.dma_start(qtf[:], qtf_d.rearrange("p (a n) -> p a n", a=KT))
                    nc.sync.dma_start(k2o[:], k2o_d.rearrange("p (a n) -> p a n", a=KT))
                    nc.sync.dma_start(agl_t[:], aglt_d[:, :])
                    nc.sync.dma_start(agr_t[:], agrt_d[:, :])
                    im2m = pm.tile([128, N], f32)
                    kthm = pm.tile([128, N], f32)
                    st_r = st_out.rearrange("(b r) n -> r b n", r=2)
                    nc.sync.dma_start(
                        im2m[:], st_r[0:1, :, :].partition_broadcast(128))
                    nc.sync.dma_start(
                        kthm[:], st_r[1:2, :, :].partition_broadcast(128))
                    def p3_head(rt):
                        r0, r1 = rt * 128, (rt + 1) * 128
                        nsqt = p3.tile([128, N], f32, tag="big",
                                       name=f"nsqt{rt}")
                        psg = ps3.tile([128, N], f32, tag="psg",
                                       name=f"p3g{rt}")
                        for nb in range(N // 512):
                            pslc = psg[:, nb * 512:(nb + 1) * 512]
                            for kt in range(KT):
                                nc.tensor.matmul(
                                    pslc, k2o[:, kt, r0:r1],
                                    qtf[:, kt, nb * 512:(nb + 1) * 512],
                                    start=(kt == 0), stop=False)
                            nc.tensor.matmul(
                                pslc, agl_t[:, r0:r1],
                                agr_t[:, nb * 512:(nb + 1) * 512],
                                start=False, stop=True)
                        nc.scalar.copy(nsqt[:], psg[:])
                        aback = p3.tile([128, N], f32, tag="big",
                                        name=f"aback{rt}")
                        nc.sync.dma_start(aback[:], a_own[r0:r1, :])
                        # u2n = sq * im2 (free-dim im2), afft = exp(-u2n)
                        u2 = p3.tile([128, N], f32, tag="big",
                                     name=f"u2_{rt}")
                        nc.vector.scalar_tensor_tensor(
                            u2[:], nsqt[:], -1.0, im2m[:],
                            op0=OP.mult, op1=OP.mult)
                        afft = p3.tile([128, N], f32, tag="big",
                                       name=f"afft{rt}")
                        nc.scalar.activation(afft[:], u2[:], AF.Exp,
                                             scale=-1.0)
                        ge = p3.tile([128, N], f32, tag="big",
                                     name=f"ge{rt}")
                        nc.vector.tensor_tensor(ge[:], afft[:], kthm[:],
                                                OP.is_ge)
                        return rt, ge, afft, aback

                    def p3_tail(st):
                        rt, ge, afft, aback = st
                        r0, r1 = rt * 128, (rt + 1) * 128
                        nc.gpsimd.tensor_tensor(afft[:], ge[:], afft[:],
                                                OP.mult)
                        nc.gpsimd.tensor_tensor(aback[:], aback[:], afft[:],
                                                OP.add)
                        # diag fixup: S_diag <- 2 (active mask picks the half)
                        for eye2, eyem, base in ((eye2a, eyema, 0),
                                                 (eye2b, eyemb, HALF)):
                            dslc = aback[:, base + rt * 128: base + (rt + 1) * 128]
                            tmp = pbs.tile([128, 128], f32, tag="dtmp",
                                           name=f"dtmp{rt}_{base}")
                            nc.gpsimd.tensor_tensor(tmp[:], dslc, eyem[:],
                                                    OP.mult)
                            nc.gpsimd.tensor_tensor(dslc, tmp[:], eye2[:],
                                                    OP.add)
                        e_t = p3.tile([128, N], f32r, tag="big",
                                      name=f"e_t{rt}")
                        z_t = pbs.tile([128, 1], f32, tag="z_t",
                                       name=f"z_t{rt}")
                        nc.scalar.activation(e_t[:], aback[:], AF.Exp,
                                             bias=bneg2[:, 0:1],
                                             accum_out=z_t[:, 0:1])
                        nc.sync.dma_start(e_own[r0:r1, :], e_t[:])
                        nc.sync.dma_start(z_own[r0:r1, :], z_t[:])

                    pend = None
                    for rt in range(RT):
                        st = p3_head(rt)
                        if pend is not None:
                            p3_tail(pend)
                        pend = st
                    p3_tail(pend)

            # ================= P4: E/Z allgather =======================
            if sim:
                nc.sync.dma_start(e_full[0:HALF, :], e_own[:, :])
                nc.sync.dma_start(e_full[HALF:N, :], e_own[:, :])
                nc.sync.dma_start(z_full[0:HALF, :], z_own[:, :])
                nc.sync.dma_start(z_full[HALF:N, :], z_own[:, :])
            else:
                nc.gpsimd.collective_compute(
                    "AllGather", OP.bypass, replica_groups=PAIRS,
                    ins=[e_own.opt()], outs=[e_full.opt()])
                nc.gpsimd.collective_compute(
                    "AllGather", OP.bypass, replica_groups=PAIRS,
                    ins=[z_own.opt()], outs=[z_full.opt()])

            # ================= P5: power iterations ====================
            MT = N // 128   # 32
            with (
                tc.tile_pool(name="pz", bufs=1) as pz,
                tc.tile_pool(name="pv", bufs=1) as pv,
                tc.tile_pool(name="pe", bufs=2) as pe,
                tc.tile_pool(name="po", bufs=3) as po,
                tc.tile_pool(name="ps5", bufs=8, space="PSUM") as ps5,
            ):
                izt = pz.tile([128, MT], f32)
                nc.sync.dma_start(
                    izt[:], z_full.rearrange("(m p) o -> p (m o)", p=128))
                iz = pz.tile([128, MT], f32)
                nc.vector.reciprocal(iz[:], izt[:])
                izq = pz.tile([128, MT], f32)
                nc.vector.tensor_scalar_mul(izq[:], iz[:], 0.25)

                vt = [pv.tile([128, VCOL], f32r, tag=f"vt{k}", name=f"vt{k}")
                      for k in range(MT)]
                ef_r = e_full.rearrange("(kb p) m -> p kb m", p=128)
                vbufs = [vbuf0, vbuf1]
                for t in range(T_POWER):
                    if t == 0:
                        src = xcol.ap().rearrange("(k p) n -> k p n", p=128)
                        for k in range(MT):
                            vf = po.tile([128, VCOL], f32, tag="vf",
                                         name=f"vf{k}")
                            nc.sync.dma_start(vf[:], src[k, :, :])
                            nc.vector.tensor_copy(vt[k][:], vf[:])
                    else:
                        src = vbufs[t % 2].rearrange("(k p) n -> k p n", p=128)
                        for k in range(MT):
                            nc.sync.dma_start(vt[k][:], src[k, :, :])
                    dst = out if t == T_POWER - 1 else vbufs[(t + 1) % 2]
                    scale = izq if t == T_POWER - 1 else iz
                    odt = f32 if t == T_POWER - 1 else f32r
                    for m in range(MT):
                        esl = pe.tile([128, MT, 128], f32r, tag="esl",
                                      name=f"esl{t}_{m}")
                        nc.sync.dma_start(
                            esl[:], ef_r[:, :, m * 128:(m + 1) * 128])
                        vo = po.tile([128, VCOL], odt, tag="vo",
                                     name=f"vo{t}_{m}")
                        for nbv in range(VCOL // 512):
                            psv = ps5.tile([128, 512], f32, tag="psv",
                                           name=f"psv{t}_{m}_{nbv}")
                            for kb in range(MT):
                                nc.tensor.matmul(
                                    psv[:], esl[:, kb, :],
                                    vt[kb][:, nbv * 512:(nbv + 1) * 512],
                                    start=(kb == 0), stop=(kb == MT - 1))
                            nc.scalar.activation(
                                vo[:, nbv * 512:(nbv + 1) * 512], psv[:],
                                AF.Copy, scale=scale[:, m:m + 1])
                        if t == T_POWER - 1:
                            nc.sync.dma_start(
                                out[m * 128:(m + 1) * 128, :], vo[:])
                        else:
                            nc.sync.dma_start(
                                dst[m * 128:(m + 1) * 128, :], vo[:])

    nc.compile()
    return nc


def _get_nc():
    if "nc" not in _CACHE:
        _CACHE["nc"] = _build()
    return _CACHE["nc"]


def _in_maps(inputs):
    X = np.ascontiguousarray(inputs["input_tensor"], dtype=np.float32)
    Wq = np.asarray(inputs["Wq"], dtype=np.float32)
    bq = np.asarray(inputs["bq"], dtype=np.float32)
    Wk = np.asarray(inputs["Wk"], dtype=np.float32)
    bk = np.asarray(inputs["bk"], dtype=np.float32)
    xt_full = np.ascontiguousarray(X.T)
    eye = np.eye(128, dtype=np.float32)
    ones = np.ones((128, 128), np.float32)
    maps = []
    for c in range(NCORES):
        h, half = c // 2, c % 2
        rows = slice(half * HALF, (half + 1) * HALF)
        cols = slice(half * VCOL, (half + 1) * VCOL)
        on = 1.0 if half == 0 else 0.0
        maps.append({
            "xt": xt_full,
            "xt_own": np.ascontiguousarray(X[rows, :].T),
            "xcol": np.ascontiguousarray(X[:, cols]),
            "wqt": np.ascontiguousarray(Wq[h].T),
            "wkt": np.ascontiguousarray(Wk[h].T),
            "bqc": np.ascontiguousarray(bq[h].reshape(HID, 1)),
            "bkc": np.ascontiguousarray(bk[h].reshape(HID, 1)),
            "e2a": 2.0 * on * eye,
            "ema": ones - on * eye,
            "e2b": 2.0 * (1.0 - on) * eye,
            "emb": ones - (1.0 - on) * eye,
        })
    return maps


def _run(inputs, trace=False):
    from concourse.bass_utils import run_bass_kernel_spmd
    nc = _get_nc()
    res = run_bass_kernel_spmd(nc, _in_maps(inputs),
                               core_ids=list(range(NCORES)), trace=trace)
    outp = np.zeros((N, D), dtype=np.float32)
    for c in range(NCORES):
        half = c % 2
        cols = slice(half * VCOL, (half + 1) * VCOL)
        outp[:, cols] += res.results[c]["out"]
    return outp, res


def kernel(**inputs):
    outp, _ = _run(inputs)
    return outp



# revision 2
# speedup vs baseline: 1.1607x; 1.1607x over previous
"""Trainium2 Bass kernel for nn_Cell2Cell (retrieval_knn, 4-head Markov power).

Sharding: head-parallel x row-parallel. Core c -> head h=c//2, half=c%2.
Per core: per-head projections (fp32r matmuls), row-block distance matrix via
augmented-gram matmul, exact per-row rank-11/rank-30 selection with DVE
max8+match_replace, knn mask in aff domain, symmetrization via a transposed-
gram pass. The Markov matrix is stored SHIFTED in fp8: A = exp(S) - 1 (so the
all-ones background J of exp(S) is carried analytically), with a second /16
"lo" copy for the final two power iterations' V-residual planes. Power
iterations run as fp8 DoubleRow matmuls (4x PE rate) with V resident in SBUF
as fp8 hi(+lo) ping-pong planes; the J-background term is folded back in as a
colsum correction row per psum; eviction scale 1/sum(exp(S)) completes
P = E/Z. Host sums head partials for the mean.
"""
import sys
sys.path.insert(0, '/opt/trn_rl_repo')
import numpy as np

N = 4096
D = 2048
HID = 256
HEADS = 4
T_POWER = 6
NCORES = 8
HALF = N // 2          # 2048 rows per core
VCOL = D // 2          # 1024 V-columns per core
RT = HALF // 128       # 16 row tiles per core
KT = HID // 128        # 2 hidden k-tiles
DKT = D // 128         # 16 input-dim k-tiles
LO_ITERS = (0, 1, 2, 3, 4, 5)  # iterations whose V carries a x16 residual plane

_CACHE = {}


def _build(sim=False):
    import concourse.bacc as bacc
    import concourse.mybir as mybir
    import concourse.tile as tile

    dt = mybir.dt
    AF = mybir.ActivationFunctionType
    OP = mybir.AluOpType
    PM = mybir.MatmulPerfMode

    nc = bacc.Bacc("TRN2", target_bir_lowering=False, debug=False,
                   num_devices=1 if sim else NCORES)

    f32, f32r, f8 = dt.float32, dt.float32r, dt.float8e4

    # ---------------- I/O ----------------
    xt = nc.dram_tensor("xt", [D, N], f32, kind="ExternalInput")        # X.T
    xt_own = nc.dram_tensor("xt_own", [D, HALF], f32, kind="ExternalInput")
    xcol = nc.dram_tensor("xcol", [N, VCOL], f32, kind="ExternalInput")
    wqt = nc.dram_tensor("wqt", [D, HID], f32, kind="ExternalInput")    # Wq[h].T
    wkt = nc.dram_tensor("wkt", [D, HID], f32, kind="ExternalInput")
    bqc = nc.dram_tensor("bqc", [HID, 1], f32, kind="ExternalInput")
    bkc = nc.dram_tensor("bkc", [HID, 1], f32, kind="ExternalInput")
    e2a = nc.dram_tensor("e2a", [128, 128], f32, kind="ExternalInput")  # 2I or 0
    ema = nc.dram_tensor("ema", [128, 128], f32, kind="ExternalInput")  # 1-I or 1
    e2b = nc.dram_tensor("e2b", [128, 128], f32, kind="ExternalInput")
    emb = nc.dram_tensor("emb", [128, 128], f32, kind="ExternalInput")
    out = nc.dram_tensor("out", [N, VCOL], f32, kind="ExternalOutput")

    PAIRS = [[0, 1], [2, 3], [4, 5], [6, 7]]

    with tile.TileContext(nc) as tc:
        with (
            tc.tile_pool(name="persist", bufs=1) as pp,
            tc.tile_pool(name="dram", bufs=1, space="DRAM") as dram,
        ):
            # ---- persistent DRAM buffers ----
            a_own = dram.tile([HALF, N], f32)            # masked affA rows
            eh_own = dram.tile([HALF, N], f8)            # A = exp(S)-1
            el_own = dram.tile([HALF, N], f8)            # A/16
            eh_full = dram.tile([N, N], f8)
            el_full = dram.tile([N, N], f8)
            st_in = dram.tile([2, HALF], f32)            # [invmd2; kth]
            st_out = dram.tile([4, HALF], f32)
            z_own = dram.tile([HALF, 1], f32)            # sum(exp(S)) rows
            z_full = dram.tile([N, 1], f32)

            # ---- small persistent SBUF ----
            b1e10 = pp.tile([128, 1], f32)
            nc.vector.memset(b1e10[:], 1e-10)
            ones_f = pp.tile([128, 1], f32)
            nc.vector.memset(ones_f[:], 1.0)
            ones_l = pp.tile([128, 1], f32r)
            nc.vector.tensor_copy(ones_l[:], ones_f[:])
            eye2a = pp.tile([128, 128], f32)
            eyema = pp.tile([128, 128], f32)
            eye2b = pp.tile([128, 128], f32)
            eyemb = pp.tile([128, 128], f32)
            nc.sync.dma_start(eye2a[:], e2a[:, :])
            nc.sync.dma_start(eyema[:], ema[:, :])
            nc.sync.dma_start(eye2b[:], e2b[:, :])
            nc.sync.dma_start(eyemb[:], emb[:, :])
            # fp8 constants for DoubleRow colsum rows (M=128: the colsum
            # lands replicated on every psum partition, no broadcast needed)
            ones8 = pp.tile([128, 2, 128], f8)
            nc.vector.memset(ones8[:], 1.0)
            ones8l = pp.tile([128, 2, 128], f8)
            nc.vector.memset(ones8l[:], 1.0 / 16.0)
            # f32r 1/128 block for the colsum correction row
            onesc_f = pp.tile([128, 128], f32)
            nc.vector.memset(onesc_f[:], 1.0 / 128.0)
            onesc = pp.tile([128, 128], f32r)
            nc.vector.tensor_copy(onesc[:], onesc_f[:])

            qtf_d = dram.tile([128, KT * N], f32r)
            k2o_d = dram.tile([128, KT * HALF], f32r)
            aglt_d = dram.tile([2, HALF], f32r)
            agrt_d = dram.tile([2, N], f32r)
            pjb_cm = tc.tile_pool(name="projsB", bufs=1)
            pjb = pjb_cm.__enter__()                   # live P0..P1
            if True:
                ktf = pjb.tile([128, KT, N], f32r)     # kT_full
                q2o = pjb.tile([128, KT, HALF], f32r)  # 2*qT_own
                agl_a = pjb.tile([2, HALF], f32r)      # [-qq_own; -1]
                agr_a = pjb.tile([2, N], f32r)         # [1; kk_full]
                pja_cm = tc.tile_pool(name="projsA", bufs=1)
                pja = pja_cm.__enter__()               # live P0 only (spilled)
                qtf = pja.tile([128, KT, N], f32r)     # qT_full
                k2o = pja.tile([128, KT, HALF], f32r)  # 2*kT_own
                agl_t = pja.tile([2, HALF], f32r)      # [-kk_own; -1]
                agr_t = pja.tile([2, N], f32r)         # [1; qq_full]

                # ================= P0: projections =================
                with (
                    tc.tile_pool(name="p0", bufs=2) as p0,
                    tc.tile_pool(name="p0w", bufs=1) as p0w,
                    tc.tile_pool(name="ps0", bufs=2, space="PSUM") as ps0,
                ):
                    wq_s = p0w.tile([128, DKT, HID], f32r)
                    wk_s = p0w.tile([128, DKT, HID], f32r)
                    for wsrc, wdst in ((wqt, wq_s), (wkt, wk_s)):
                        wr = wsrc.ap().rearrange("(a p) m -> p a m", p=128)
                        for ch in range(2):
                            wf = p0.tile([128, DKT // 2, HID], f32,
                                         tag="wstg", bufs=1,
                                         name=f"wf_{wdst.tensor.name}_{ch}")
                            nc.sync.dma_start(
                                wf[:], wr[:, ch * 8:(ch + 1) * 8, :])
                            nc.vector.tensor_copy(
                                wdst[:, ch * 8:(ch + 1) * 8, :], wf[:])
                    bq_s = p0w.tile([128, KT], f32)
                    bk_s = p0w.tile([128, KT], f32)
                    nc.sync.dma_start(
                        bq_s[:], bqc.ap().rearrange("(a p) o -> p (a o)", p=128))
                    nc.sync.dma_start(
                        bk_s[:], bkc.ap().rearrange("(a p) o -> p (a o)", p=128))

                    xt_r = xt.ap().rearrange("(a p) n -> p a n", p=128)
                    xto_r = xt_own.ap().rearrange("(a p) n -> p a n", p=128)

                    def proj(nb, rhs_src, pairs):
                        # kk-outer: one rhs k-tile shared by all 4 psums
                        psms = []
                        for w_s, b_s, dst, scaled in pairs:
                            for mt in range(KT):
                                psms.append(ps0.tile(
                                    [128, 512], f32, tag=f"psm{len(psms)}",
                                    name=f"psm{nb}_{len(psms)}"))
                        for kk in range(DKT):
                            slf = p0.tile([128, 512], f32, tag="rhsf",
                                          bufs=3, name=f"rhsf{nb}_{kk}")
                            nc.sync.dma_start(
                                slf[:], rhs_src[:, kk,
                                                nb * 512:(nb + 1) * 512])
                            sl = p0.tile([128, 512], f32r, tag="rhs",
                                         bufs=3, name=f"rhs{nb}_{kk}")
                            nc.vector.tensor_copy(sl[:], slf[:])
                            i = 0
                            for w_s, b_s, dst, scaled in pairs:
                                for mt in range(KT):
                                    nc.tensor.matmul(
                                        psms[i],
                                        w_s[:, kk, mt * 128:(mt + 1) * 128],
                                        sl[:],
                                        start=(kk == 0), stop=(kk == DKT - 1))
                                    i += 1
                        i = 0
                        for w_s, b_s, dst, scaled in pairs:
                            for mt in range(KT):
                                if scaled:
                                    nc.vector.tensor_scalar(
                                        dst[:, mt, nb * 512:(nb + 1) * 512],
                                        psms[i], b_s[:, mt:mt + 1], 2.0,
                                        OP.add, OP.mult)
                                else:
                                    nc.vector.tensor_scalar_add(
                                        dst[:, mt, nb * 512:(nb + 1) * 512],
                                        psms[i], b_s[:, mt:mt + 1])
                                i += 1

                    for nb in range(N // 512):
                        proj(nb, xt_r, ((wq_s, bq_s, qtf, False),
                                        (wk_s, bk_s, ktf, False)))
                    for nb in range(HALF // 512):
                        proj(nb, xto_r, ((wq_s, bq_s, q2o, True),
                                         (wk_s, bk_s, k2o, True)))

                # ---- norms via ones-matmul over squared projections ----
                with (
                    tc.tile_pool(name="pn", bufs=1) as pn,
                    tc.tile_pool(name="psn", bufs=4, space="PSUM") as psn,
                ):
                    trow = pn.tile([1, 512], f32r, tag="trow")
                    cm = pn.tile([2, N], f32, tag="cm")
                    nc.vector.memset(cm[:, :], -1.0)
                    nc.vector.tensor_copy(agl_a[:, :], cm[:, :HALF])
                    nc.vector.tensor_copy(agl_t[:, :], cm[:, :HALF])
                    nc.vector.memset(cm[:, :], 1.0)
                    nc.vector.tensor_copy(agr_a[:, :], cm[:, :])
                    nc.vector.tensor_copy(agr_t[:, :], cm[:, :])
                    for src, aug, row, sgn, w in (
                        (ktf, agr_a, 1, 1.0, N),       # +kk_full
                        (qtf, agr_t, 1, 1.0, N),       # +qq_full
                        (q2o, agl_a, 0, -0.25, HALF),  # -qq_own (q2o = 2q)
                        (k2o, agl_t, 0, -0.25, HALF),  # -kk_own
                    ):
                        sq = pn.tile([128, KT, N], f32r, tag="sq",
                                     name=f"sq_{aug.tensor.name}_{row}")
                        nc.vector.tensor_tensor(
                            sq[:, :, :w], src[:, :, :w], src[:, :, :w], OP.mult)
                        for nb in range(w // 512):
                            pst = psn.tile([1, 512], f32, tag="pst",
                                           name=f"pst{nb}")
                            for kt in range(KT):
                                nc.tensor.matmul(
                                    pst[:], ones_l[:],
                                    sq[:, kt, nb * 512:(nb + 1) * 512],
                                    start=(kt == 0), stop=(kt == KT - 1))
                            if row == 0:
                                nc.vector.tensor_scalar_mul(
                                    aug[0:1, nb * 512:(nb + 1) * 512], pst[:], sgn)
                            else:
                                tr = pn.tile([1, 512], f32r, tag="trow",
                                             name=f"tr_{aug.tensor.name}_{nb}")
                                nc.vector.tensor_scalar_mul(tr[:], pst[:], sgn)
                                nc.sync.dma_start(
                                    aug[1:2, nb * 512:(nb + 1) * 512], tr[:])

                # ---- spill P3-only tensors, free their SBUF ----
                nc.sync.dma_start(qtf_d[:, :], qtf.rearrange("p a n -> p (a n)"))
                nc.sync.dma_start(k2o_d[:, :], k2o.rearrange("p a n -> p (a n)"))
                nc.sync.dma_start(aglt_d[:, :], agl_t[:, :])
                nc.sync.dma_start(agrt_d[:, :], agr_t[:, :])
                pja_cm.__exit__(None, None, None)

                # ================= P1: A-side rows + stats =================
                with (
                    tc.tile_pool(name="big1", bufs=8) as pb,
                    tc.tile_pool(name="pbs1", bufs=2) as pbs,
                    tc.tile_pool(name="ps1", bufs=1, space="PSUM") as ps1,
                ):
                    p1, p1s = pb, pbs
                    prev = None  # (msk, im2, kth, r0, r1) delayed by one tile
                    for rt in range(RT):
                        r0, r1 = rt * 128, (rt + 1) * 128
                        nsq = p1.tile([128, N], f32, tag="big",
                                      name=f"nsq{rt}")
                        psg = ps1.tile([128, N], f32, tag="psg",
                                       name=f"psg{rt}")
                        for nb in range(N // 512):
                            pslc = psg[:, nb * 512:(nb + 1) * 512]
                            for kt in range(KT):
                                nc.tensor.matmul(
                                    pslc, q2o[:, kt, r0:r1],
                                    ktf[:, kt, nb * 512:(nb + 1) * 512],
                                    start=(kt == 0), stop=False)
                            nc.tensor.matmul(
                                pslc, agl_a[:, r0:r1],
                                agr_a[:, nb * 512:(nb + 1) * 512],
                                start=False, stop=True)
                        nc.scalar.copy(nsq[:], psg[:])
                        # exact 32 smallest sq = 32 largest of nsq (=-sq)
                        sel = p1s.tile([128, 32], f32, tag="sel",
                                       name=f"sel{rt}")
                        sca = p1.tile([128, N], f32, tag="big",
                                      name=f"sca{rt}")
                        nc.vector.max(sel[:, 0:8], nsq[:])
                        nc.vector.match_replace(sca[:], sel[:, 0:8], nsq[:],
                                                -1e30)
                        scb = p1.tile([128, N], f32, tag="big",
                                      name=f"scb{rt}")
                        nc.vector.max(sel[:, 8:16], sca[:])
                        nc.vector.match_replace(scb[:], sel[:, 8:16], sca[:],
                                                -1e30)
                        scc = p1.tile([128, N], f32, tag="big",
                                      name=f"scc{rt}")
                        nc.vector.max(sel[:, 16:24], scb[:])
                        nc.vector.match_replace(scc[:], sel[:, 16:24], scb[:],
                                                -1e30)
                        nc.vector.max(sel[:, 24:32], scc[:])
                        # stats on DVE: im2 = 1/relu(sq11), kth = exp(-sq30*im2)
                        t11 = p1s.tile([128, 1], f32, tag="t11",
                                       name=f"t11{rt}")
                        nc.vector.tensor_scalar(t11[:], sel[:, 10:11], -1.0,
                                                1e-20, OP.mult, OP.max)
                        im2 = p1s.tile([128, 1], f32, tag="im2",
                                       name=f"im2{rt}")
                        nc.vector.reciprocal(im2[:], t11[:])
                        kth = p1s.tile([128, 1], f32, tag="kth",
                                       name=f"kth{rt}")
                        nc.scalar.activation(kth[:], sel[:, 29:30], AF.Exp,
                                             scale=im2[:, 0:1])
                        # aff = exp(nsq * im2)   (nsq = -sq)
                        aff = p1.tile([128, N], f32, tag="big",
                                      name=f"aff{rt}")
                        nc.scalar.activation(aff[:], nsq[:], AF.Exp,
                                             scale=im2[:, 0:1])
                        if prev is not None:
                            paff, pim2, pkth, pr0, pr1 = prev
                            pmsk = p1.tile([128, N], f32, tag="big",
                                           name=f"msk{rt - 1}")
                            nc.vector.scalar_tensor_tensor(
                                pmsk[:], paff[:], pkth[:, 0:1], paff[:],
                                op0=OP.is_ge, op1=OP.mult)
                            nc.sync.dma_start(a_own[pr0:pr1, :], pmsk[:])
                            nc.sync.dma_start(st_in[0:1, pr0:pr1], pim2[:])
                            nc.sync.dma_start(st_in[1:2, pr0:pr1], pkth[:])
                        prev = (aff, im2, kth, r0, r1)
                    paff, pim2, pkth, pr0, pr1 = prev
                    pmsk = p1.tile([128, N], f32, tag="big", name="msk_last")
                    nc.vector.scalar_tensor_tensor(
                        pmsk[:], paff[:], pkth[:, 0:1], paff[:],
                        op0=OP.is_ge, op1=OP.mult)
                    nc.sync.dma_start(a_own[pr0:pr1, :], pmsk[:])
                    nc.sync.dma_start(st_in[0:1, pr0:pr1], pim2[:])
                    nc.sync.dma_start(st_in[1:2, pr0:pr1], pkth[:])

                pjb_cm.__exit__(None, None, None)

                # ============ P2: stats allgather + bcast mats ============
                if sim:
                    nc.sync.dma_start(st_out[0:2, :], st_in[:, :])
                    nc.sync.dma_start(st_out[2:4, :], st_in[:, :])
                else:
                    nc.gpsimd.collective_compute(
                        "AllGather", OP.bypass, replica_groups=PAIRS,
                        ins=[st_in.opt()], outs=[st_out.opt()])

                # ======== P3: AT-side + S + A=exp(S)-1 (fp8 hi/lo) ========
                with (
                    tc.tile_pool(name="rl", bufs=1) as rl,
                    tc.tile_pool(name="mats", bufs=1) as pm,
                    tc.tile_pool(name="big3", bufs=5) as pb3,
                    tc.tile_pool(name="pbs3", bufs=2) as pbs,
                    tc.tile_pool(name="pf8", bufs=2) as pf8,
                    tc.tile_pool(name="ps3", bufs=1, space="PSUM") as ps3,
                ):
                    p3 = pb3
                    qtf = rl.tile([128, KT, N], f32r)
                    k2o = rl.tile([128, KT, HALF], f32r)
                    agl_t = rl.tile([2, HALF], f32r)
                    agr_t = rl.tile([2, N], f32r)
                    nc.sync.dma_start(qtf[:], qtf_d.rearrange("p (a n) -> p a n", a=KT))
                    nc.sync.dma_start(k2o[:], k2o_d.rearrange("p (a n) -> p a n", a=KT))
                    nc.sync.dma_start(agl_t[:], aglt_d[:, :])
                    nc.sync.dma_start(agr_t[:], agrt_d[:, :])
                    im2m = pm.tile([128, N], f32)
                    kthm = pm.tile([128, N], f32)
                    st_r = st_out.rearrange("(b r) n -> r b n", r=2)
                    nc.sync.dma_start(
                        im2m[:], st_r[0:1, :, :].partition_broadcast(128))
                    nc.sync.dma_start(
                        kthm[:], st_r[1:2, :, :].partition_broadcast(128))

                    def p3_head(rt):
                        r0, r1 = rt * 128, (rt + 1) * 128
                        nsqt = p3.tile([128, N], f32, tag="big",
                                       name=f"nsqt{rt}")
                        psg = ps3.tile([128, N], f32, tag="psg",
                                       name=f"p3g{rt}")
                        for nb in range(N // 512):
                            pslc = psg[:, nb * 512:(nb + 1) * 512]
                            for kt in range(KT):
                                nc.tensor.matmul(
                                    pslc, k2o[:, kt, r0:r1],
                                    qtf[:, kt, nb * 512:(nb + 1) * 512],
                                    start=(kt == 0), stop=False)
                            nc.tensor.matmul(
                                pslc, agl_t[:, r0:r1],
                                agr_t[:, nb * 512:(nb + 1) * 512],
                                start=False, stop=True)
                        nc.scalar.copy(nsqt[:], psg[:])
                        aback = p3.tile([128, N], f32, tag="big",
                                        name=f"aback{rt}")
                        nc.sync.dma_start(aback[:], a_own[r0:r1, :])
                        # u2n = sq * im2 (free-dim im2), afft = exp(-u2n)
                        u2 = p3.tile([128, N], f32, tag="big",
                                     name=f"u2_{rt}")
                        nc.vector.scalar_tensor_tensor(
                            u2[:], nsqt[:], -1.0, im2m[:],
                            op0=OP.mult, op1=OP.mult)
                        afft = p3.tile([128, N], f32, tag="big",
                                       name=f"afft{rt}")
                        nc.scalar.activation(afft[:], u2[:], AF.Exp,
                                             scale=-1.0)
                        nc.vector.tensor_tensor(u2[:], afft[:], kthm[:],
                                                OP.is_ge)
                        return rt, u2, afft, aback

                    def p3_tail(st):
                        rt, ge, afft, aback = st
                        r0, r1 = rt * 128, (rt + 1) * 128
                        nc.gpsimd.tensor_tensor(afft[:], ge[:], afft[:],
                                                OP.mult)
                        nc.gpsimd.tensor_tensor(aback[:], aback[:], afft[:],
                                                OP.add)
                        # diag fixup: S_diag <- 2 (active mask picks the half)
                        for eye2, eyem, base in ((eye2a, eyema, 0),
                                                 (eye2b, eyemb, HALF)):
                            dslc = aback[:, base + rt * 128: base + (rt + 1) * 128]
                            tmp = pbs.tile([128, 128], f32, tag="dtmp",
                                           name=f"dtmp{rt}_{base}")
                            nc.gpsimd.tensor_tensor(tmp[:], dslc, eyem[:],
                                                    OP.mult)
                            nc.gpsimd.tensor_tensor(dslc, tmp[:], eye2[:],
                                                    OP.add)
                        # es = exp(S) in f32 with z accumulation
                        es = p3.tile([128, N], f32, tag="big",
                                     name=f"es{rt}")
                        z_t = pbs.tile([128, 1], f32, tag="z_t",
                                       name=f"z_t{rt}")
                        nc.scalar.activation(es[:], aback[:], AF.Exp,
                                             accum_out=z_t[:, 0:1])
                        # A = es - 1 (fp8 hi), A/16 (fp8 lo)
                        ah = pf8.tile([128, N], f8, tag="ah",
                                      bufs=2, name=f"ah{rt}")
                        nc.vector.tensor_scalar_add(ah[:], es[:], -1.0)
                        al = pf8.tile([128, N], f8, tag="al",
                                      bufs=2, name=f"al{rt}")
                        nc.vector.tensor_scalar(al[:], es[:], -1.0,
                                                1.0 / 16.0, OP.add, OP.mult)
                        nc.sync.dma_start(eh_own[r0:r1, :], ah[:])
                        nc.sync.dma_start(el_own[r0:r1, :], al[:])
                        nc.sync.dma_start(z_own[r0:r1, :], z_t[:])

                    pend = None
                    for rt in range(RT):
                        st = p3_head(rt)
                        if pend is not None:
                            p3_tail(pend)
                        pend = st
                    p3_tail(pend)

            # ================= P4: E/Z allgather =======================
            if sim:
                nc.sync.dma_start(eh_full[0:HALF, :], eh_own[:, :])
                nc.sync.dma_start(eh_full[HALF:N, :], eh_own[:, :])
                nc.sync.dma_start(el_full[0:HALF, :], el_own[:, :])
                nc.sync.dma_start(el_full[HALF:N, :], el_own[:, :])
                nc.sync.dma_start(z_full[0:HALF, :], z_own[:, :])
                nc.sync.dma_start(z_full[HALF:N, :], z_own[:, :])
            else:
                nc.gpsimd.collective_compute(
                    "AllGather", OP.bypass, replica_groups=PAIRS,
                    ins=[eh_own.opt()], outs=[eh_full.opt()])
                nc.gpsimd.collective_compute(
                    "AllGather", OP.bypass, replica_groups=PAIRS,
                    ins=[el_own.opt()], outs=[el_full.opt()])
                nc.gpsimd.collective_compute(
                    "AllGather", OP.bypass, replica_groups=PAIRS,
                    ins=[z_own.opt()], outs=[z_full.opt()])

            # ========== P5: fp8 DoubleRow power iterations =============
            MT = N // 128   # 32
            MG = 2          # m-tiles per E-stream group
            with (
                tc.tile_pool(name="pz", bufs=1) as pz,
                tc.tile_pool(name="pv", bufs=1) as pv,
                tc.tile_pool(name="pe", bufs=2) as pe,
                tc.tile_pool(name="po", bufs=3) as po,
                tc.tile_pool(name="pcs", bufs=2) as pcs,
                tc.tile_pool(name="ps5", bufs=4, space="PSUM") as ps5,
                tc.tile_pool(name="psc", bufs=2, space="PSUM") as psc,
            ):
                izt = pz.tile([128, MT], f32)
                nc.sync.dma_start(
                    izt[:], z_full.rearrange("(m p) o -> p (m o)", p=128))
                iz = pz.tile([128, MT], f32)
                nc.vector.reciprocal(iz[:], izt[:])
                izq = pz.tile([128, MT], f32)
                nc.vector.tensor_scalar_mul(izq[:], iz[:], 0.25)
                iz16 = pz.tile([128, MT], f32)
                nc.vector.tensor_scalar_mul(iz16[:], iz[:], 16.0)

                # V planes: hi for every iteration, lo (x16 residual) planes
                # ping-pong for the LO_ITERS inputs
                vh = [pv.tile([128, MT, VCOL], f8, tag=f"vh{k}",
                              name=f"vh{k}") for k in range(2)]
                vl = [pv.tile([128, MT, VCOL], f8, tag=f"vl{k}",
                              name=f"vl{k}") for k in range(2)]

                # V0 = quantized X columns; hi plane + x16 residual lo plane
                with tc.tile_pool(name="pstg", bufs=2) as pstg:
                    src0 = xcol.ap().rearrange("(k p) n -> k p n", p=128)
                    for k in range(MT):
                        vf = pstg.tile([128, VCOL], f32, tag="vf",
                                       name=f"vf{k}")
                        nc.sync.dma_start(vf[:], src0[k, :, :])
                        nc.vector.tensor_copy(vh[0][:, k, :], vf[:])
                        if 0 in LO_ITERS:
                            vf16 = pstg.tile([128, VCOL], f32, tag="vf16",
                                             name=f"vf16_{k}")
                            nc.vector.tensor_scalar_mul(vf16[:], vf[:], 16.0)
                            nc.vector.scalar_tensor_tensor(
                                vl[0][:, k, :], vh[0][:, k, :], -16.0,
                                vf16[:], op0=OP.mult, op1=OP.add)

                ehr = eh_full.rearrange("(kb p) m -> p kb m", p=128)
                elr = el_full.rearrange("(kb p) m -> p kb m", p=128)

                for t in range(T_POWER):
                    lo = t in LO_ITERS          # input carries lo plane
                    mk_lo = (t + 1) in LO_ITERS and t + 1 < T_POWER
                    srch = vh[t % 2]
                    srcl = vl[t % 2]
                    dsth = vh[(t + 1) % 2]
                    dstl = vl[(t + 1) % 2]
                    last = t == T_POWER - 1
                    scale = izq if last else iz

                    # ---- colsum of V-hat (exact, via ones DoubleRow; the
                    # all-ones M=128 lhsT replicates the colsum on every
                    # psum partition) ----
                    cs_bc = pcs.tile([128, VCOL], f32r, tag="csbc",
                                     name=f"csbc{t}")
                    for nbv in range(VCOL // 512):
                        ps_cs = psc.tile([128, 512], f32, tag="pscs",
                                         name=f"pscs{t}_{nbv}")
                        planes = [(ones8, srch)]
                        if lo:
                            planes.append((ones8l, srcl))
                        njt = len(planes) * (MT // 2)
                        ji = 0
                        for w8, vsrc in planes:
                            for j in range(MT // 2):
                                nc.tensor.matmul(
                                    ps_cs[:], w8[:, :, :],
                                    vsrc[:, 2 * j:2 * j + 2,
                                         nbv * 512:(nbv + 1) * 512],
                                    start=(ji == 0), stop=(ji == njt - 1),
                                    perf_mode=PM.DoubleRow)
                                ji += 1
                        nc.scalar.copy(
                            cs_bc[:, nbv * 512:(nbv + 1) * 512], ps_cs[:])

                    for m in range(MT):
                        g0 = (m // MG) * MG
                        if m % MG == 0:
                            eslh = pe.tile([128, MT, MG * 128], f8,
                                           tag="eslh", name=f"eslh{t}_{m}")
                            nc.sync.dma_start(
                                eslh[:], ehr[:, :, g0 * 128:(g0 + MG) * 128])
                            if lo:
                                esll = pe.tile([128, MT, MG * 128], f8,
                                               tag="esll",
                                               name=f"esll{t}_{m}")
                                nc.sync.dma_start(
                                    esll[:],
                                    elr[:, :, g0 * 128:(g0 + MG) * 128])
                            cur_eh, cur_el = eslh, (esll if lo else None)
                        mo = (m - g0) * 128
                        for nbv in range(VCOL // 512):
                            psv = ps5.tile([128, 512], f32, tag="psv",
                                           name=f"psv{t}_{m}_{nbv}")
                            for j in range(MT // 2):
                                nc.tensor.matmul(
                                    psv[:],
                                    cur_eh[:, 2 * j:2 * j + 2, mo:mo + 128],
                                    srch[:, 2 * j:2 * j + 2,
                                         nbv * 512:(nbv + 1) * 512],
                                    start=(j == 0), stop=False,
                                    perf_mode=PM.DoubleRow)
                            if lo:
                                for j in range(MT // 2):
                                    nc.tensor.matmul(
                                        psv[:],
                                        cur_el[:, 2 * j:2 * j + 2,
                                               mo:mo + 128],
                                        srcl[:, 2 * j:2 * j + 2,
                                             nbv * 512:(nbv + 1) * 512],
                                        start=False, stop=False,
                                        perf_mode=PM.DoubleRow)
                            # J-background correction row: += colsum
                            nc.tensor.matmul(
                                psv[:], onesc[:, :],
                                cs_bc[:, nbv * 512:(nbv + 1) * 512],
                                start=False, stop=True,
                                skip_group_check=True)
                            if last:
                                vo = po.tile([128, 512], f32, tag="vo",
                                             name=f"vo{t}_{m}_{nbv}")
                                nc.scalar.activation(
                                    vo[:], psv[:], AF.Copy,
                                    scale=scale[:, m:m + 1])
                                nc.sync.dma_start(
                                    out[m * 128:(m + 1) * 128,
                                        nbv * 512:(nbv + 1) * 512], vo[:])
                            elif mk_lo:
                                # next iter needs hi+lo: evict 16*V, derive
                                # Vh = (16V)/16 (fp8 RTN) and lo = 16V - 16*Vh
                                vo = po.tile([128, 512], f32, tag="vo",
                                             name=f"vo{t}_{m}_{nbv}")
                                nc.scalar.activation(
                                    vo[:], psv[:], AF.Copy,
                                    scale=iz16[:, m:m + 1])
                                dh = dsth[:, m, nbv * 512:(nbv + 1) * 512]
                                nc.vector.tensor_scalar_mul(
                                    dh, vo[:], 1.0 / 16.0)
                                nc.vector.scalar_tensor_tensor(
                                    dstl[:, m, nbv * 512:(nbv + 1) * 512],
                                    dh, -16.0, vo[:],
                                    op0=OP.mult, op1=OP.add)
                            else:
                                nc.scalar.activation(
                                    dsth[:, m, nbv * 512:(nbv + 1) * 512],
                                    psv[:], AF.Copy,
                                    scale=scale[:, m:m + 1])

    nc.compile()
    return nc


def _get_nc():
    if "nc" not in _CACHE:
        _CACHE["nc"] = _build()
    return _CACHE["nc"]


def _in_maps(inputs):
    X = np.ascontiguousarray(inputs["input_tensor"], dtype=np.float32)
    Wq = np.asarray(inputs["Wq"], dtype=np.float32)
    bq = np.asarray(inputs["bq"], dtype=np.float32)
    Wk = np.asarray(inputs["Wk"], dtype=np.float32)
    bk = np.asarray(inputs["bk"], dtype=np.float32)
    xt_full = np.ascontiguousarray(X.T)
    eye = np.eye(128, dtype=np.float32)
    ones = np.ones((128, 128), np.float32)
    maps = []
    for c in range(NCORES):
        h, half = c // 2, c % 2
        rows = slice(half * HALF, (half + 1) * HALF)
        cols = slice(half * VCOL, (half + 1) * VCOL)
        on = 1.0 if half == 0 else 0.0
        maps.append({
            "xt": xt_full,
            "xt_own": np.ascontiguousarray(X[rows, :].T),
            "xcol": np.ascontiguousarray(X[:, cols]),
            "wqt": np.ascontiguousarray(Wq[h].T),
            "wkt": np.ascontiguousarray(Wk[h].T),
            "bqc": np.ascontiguousarray(bq[h].reshape(HID, 1)),
            "bkc": np.ascontiguousarray(bk[h].reshape(HID, 1)),
            "e2a": 2.0 * on * eye,
            "ema": ones - on * eye,
            "e2b": 2.0 * (1.0 - on) * eye,
            "emb": ones - (1.0 - on) * eye,
        })
    return maps


def _run(inputs, trace=False):
    from concourse.bass_utils import run_bass_kernel_spmd
    nc = _get_nc()
    res = run_bass_kernel_spmd(nc, _in_maps(inputs),
                               core_ids=list(range(NCORES)), trace=trace)
    outp = np.zeros((N, D), dtype=np.float32)
    for c in range(NCORES):
        half = c % 2
        cols = slice(half * VCOL, (half + 1) * VCOL)
        outp[:, cols] += res.results[c]["out"]
    return outp, res


def kernel(**inputs):
    outp, _ = _run(inputs)
    return outp
